# revision 1
# baseline (speedup 1.0000x reference)
"""Trainium2 Bass kernel for nn_AttnBlock (GroupNorm + single-head 4096-token
attention + residual), sharded over 8 NeuronCores.

Sharding: data-parallel over batch B=4, sequence-parallel x2 over the 4096
query tokens -> 8 shards. Each core computes k/v for its full batch
(duplicated across the 2 token-halves) and q/attention/out-proj for its 2048
query tokens. The token axis is rolled on the host for the second half so a
single SPMD NEFF serves all cores (softmax over keys is order-invariant,
groupnorm stats are token-permutation-invariant).

Self-contained: hardcodes all shapes; only needs the concourse runtime.
"""

import numpy as np
import ml_dtypes

import concourse.bass as bass
import concourse.bacc as bacc
import concourse.tile as tile
from concourse import mybir
from concourse.bass_utils import run_bass_kernel_spmd

P = 128                 # partitions
C = 512                 # channels
N = 4096                # tokens (64*64)
NQ = 2048               # query tokens per core
CT = C // P             # 4 channel tiles
JT = N // P             # 32 key-token tiles of 128
NSTRIP = NQ // 512      # 4 query strips of 512
ITS = 512 // P          # 4 i-subtiles per strip
GS = 16                 # channels per group
NG = P // GS            # 8 groups per channel tile
EPS = 1e-6
SCALE = float(C) ** -0.5
F32 = mybir.dt.float32
BF16 = mybir.dt.bfloat16

_CACHE = {}


def build_bass():
    nc = bacc.Bacc(None, target_bir_lowering=False)

    x_h = nc.dram_tensor("x", [C, N], F32, kind="ExternalInput")[:]
    wq_h = nc.dram_tensor("wqT", [C, C], BF16, kind="ExternalInput")[:]
    wk_h = nc.dram_tensor("wkT", [C, C], BF16, kind="ExternalInput")[:]
    wv_h = nc.dram_tensor("wvT", [C, C], BF16, kind="ExternalInput")[:]
    wo_h = nc.dram_tensor("woT", [C, C], BF16, kind="ExternalInput")[:]
    bq_h = nc.dram_tensor("bq", [C], F32, kind="ExternalInput")[:]
    bk_h = nc.dram_tensor("bk", [C], F32, kind="ExternalInput")[:]
    bv_h = nc.dram_tensor("bv", [C], F32, kind="ExternalInput")[:]
    bo_h = nc.dram_tensor("bo", [C], F32, kind="ExternalInput")[:]
    gam_h = nc.dram_tensor("gam", [C], F32, kind="ExternalInput")[:]
    bet_h = nc.dram_tensor("bet", [C], F32, kind="ExternalInput")[:]
    out_h = nc.dram_tensor("out", [C, NQ], F32, kind="ExternalOutput")[:]

    g8_np = np.zeros((P, NG), np.float32)
    g8T_np = np.zeros((NG, P), np.float32)
    for c in range(P):
        g8_np[c, c // GS] = 1.0 / GS
        g8T_np[c // GS, c] = 1.0
    g8_h = nc.inline_tensor(g8_np, name="g8")[:]
    g8T_h = nc.inline_tensor(g8T_np, name="g8T")[:]

    x_t = x_h.rearrange("(t p) n -> t p n", p=P)          # [4,128,4096]
    out_t = out_h.rearrange("(t p) n -> t p n", p=P)      # [4,128,2048]

    def col4(ap1d):
        # [512] dram vector -> [128,4] sbuf layout (column ct holds chans ct*128..)
        return bass.AP(tensor=ap1d.tensor, offset=ap1d.offset, ap=[[1, P], [P, CT]])

    with tile.TileContext(nc) as tc:
        with tc.tile_pool(name="consts", bufs=1) as cp, \
             tc.tile_pool(name="wo", bufs=1) as wop, \
             tc.tile_pool(name="qkv", bufs=1) as qkvp, \
             tc.tile_pool(name="hT", bufs=1) as hTp, \
             tc.tile_pool(name="mm", bufs=3, space="PSUM") as mmp:

            # ---- constants ----
            ones_f32 = cp.tile([P, 1], F32, tag="ones")
            nc.vector.memset(ones_f32[:], 1.0)
            ones1 = cp.tile([1, P], F32, tag="ones1")
            nc.vector.memset(ones1[:], 1.0)
            eps_t = cp.tile([P, 1], F32, tag="eps")
            nc.vector.memset(eps_t[:], EPS)
            g8_sb = cp.tile([P, NG], F32, tag="g8")
            nc.sync.dma_start(out=g8_sb[:], in_=g8_h)
            g8T_sb = cp.tile([NG, P], F32, tag="g8T")
            nc.sync.dma_start(out=g8T_sb[:], in_=g8T_h)
            bq_sb = cp.tile([P, CT], F32, tag="bq")
            nc.sync.dma_start(out=bq_sb[:], in_=col4(bq_h))
            bk_sb = cp.tile([P, CT], F32, tag="bk")
            nc.sync.dma_start(out=bk_sb[:], in_=col4(bk_h))
            bo_sb = cp.tile([P, CT], F32, tag="bo")
            nc.sync.dma_start(out=bo_sb[:], in_=col4(bo_h))
            gam_sb = cp.tile([P, CT], F32, tag="gam")
            nc.sync.dma_start(out=gam_sb[:], in_=col4(gam_h))
            bet_sb = cp.tile([P, CT], F32, tag="bet")
            nc.sync.dma_start(out=bet_sb[:], in_=col4(bet_h))
            bv_bc = cp.tile([P, C], F32, tag="bvbc")

            # ---- persistent weights / activations ----
            wo_sb = [wop.tile([P, C], BF16, tag=f"wo{t}", name=f"wo{t}") for t in range(CT)]
            wo_t = wo_h.rearrange("(t p) o -> t p o", p=P)

            q_bf = [qkvp.tile([P, NQ], BF16, tag=f"q{t}", name=f"q{t}") for t in range(CT)]
            k_bf = [qkvp.tile([P, N], BF16, tag=f"k{t}", name=f"k{t}") for t in range(CT)]
            v_bf = [qkvp.tile([P, C], BF16, tag=f"v{j}", name=f"v{j}") for j in range(JT)]
            hT_bf = [hTp.tile([P, NQ], BF16, tag=f"hT{t}", name=f"hT{t}") for t in range(CT)]

            # =========== Phase A: groupnorm -> hn (bf16), and QKV ===========
            with tc.tile_pool(name="xg", bufs=2) as xgp, \
                 tc.tile_pool(name="gnsb", bufs=2) as gnp, \
                 tc.tile_pool(name="hn", bufs=1) as hnp, \
                 tc.tile_pool(name="wqkv", bufs=1) as wqkvp, \
                 tc.tile_pool(name="gnps", bufs=2, space="PSUM") as gnps, \
                 tc.tile_pool(name="projps", bufs=3, space="PSUM") as pjp:

                wq_sb = [wqkvp.tile([P, C], BF16, tag=f"wq{t}", name=f"wq{t}") for t in range(CT)]
                wk_sb = [wqkvp.tile([P, C], BF16, tag=f"wk{t}", name=f"wk{t}") for t in range(CT)]
                wv_sb = [wqkvp.tile([P, C], BF16, tag=f"wv{t}", name=f"wv{t}") for t in range(CT)]
                wq_t = wq_h.rearrange("(t p) o -> t p o", p=P)
                wk_t = wk_h.rearrange("(t p) o -> t p o", p=P)
                wv_t = wv_h.rearrange("(t p) o -> t p o", p=P)

                hn_bf = [hnp.tile([P, N], BF16, tag=f"hn{t}", name=f"hn{t}") for t in range(CT)]

                for ct in range(CT):
                    x_sb = xgp.tile([P, N], F32, tag="x")
                    # chunked load + per-chunk stats so stats start on the
                    # first chunk instead of after the full 2MB tile
                    stats = gnp.tile([P, 8, 6], F32, tag="stats")
                    for s in range(8):
                        nc.sync.dma_start(
                            out=x_sb[:, s * 512:(s + 1) * 512],
                            in_=x_t[ct][:, s * 512:(s + 1) * 512],
                        )
                        nc.vector.bn_stats(
                            out=stats[:, s, :], in_=x_sb[:, s * 512:(s + 1) * 512]
                        )
                    mv = gnp.tile([P, 2], F32, tag="mv")
                    nc.vector.bn_aggr(out=mv[:], in_=stats[:])
                    # cstat = [mean, E[x^2]] per channel
                    cstat = gnp.tile([P, 2], F32, tag="cstat")
                    nc.vector.tensor_copy(cstat[:, 0:1], mv[:, 0:1])
                    nc.vector.tensor_mul(cstat[:, 1:2], mv[:, 0:1], mv[:, 0:1])
                    nc.vector.tensor_add(cstat[:, 1:2], cstat[:, 1:2], mv[:, 1:2])
                    # group-average then broadcast back to channels (PE)
                    psA = gnps.tile([NG, 2], F32, tag="gn")
                    nc.tensor.matmul(psA[:], lhsT=g8_sb[:], rhs=cstat[:],
                                     start=True, stop=True)
                    gt = gnp.tile([NG, 2], F32, tag="gt")
                    nc.vector.tensor_copy(gt[:], psA[:])
                    psB = gnps.tile([P, 2], F32, tag="gn")
                    nc.tensor.matmul(psB[:], lhsT=g8T_sb[:], rhs=gt[:],
                                     start=True, stop=True)
                    gstat = gnp.tile([P, 2], F32, tag="gstat")
                    nc.vector.tensor_copy(gstat[:], psB[:])
                    # a = gamma * rsqrt(gvar+eps); d = beta - gmean * a
                    vtmp = gnp.tile([P, 1], F32, tag="vtmp")
                    nc.vector.tensor_mul(vtmp[:], gstat[:, 0:1], gstat[:, 0:1])
                    nc.vector.tensor_tensor(
                        out=vtmp[:], in0=gstat[:, 1:2], in1=vtmp[:],
                        op=mybir.AluOpType.subtract,
                    )
                    nc.scalar.activation(
                        out=vtmp[:], in_=vtmp[:],
                        func=mybir.ActivationFunctionType.Sqrt,
                        bias=eps_t[:], scale=1.0,
                    )
                    rstd = gnp.tile([P, 1], F32, tag="rstd")
                    nc.vector.reciprocal(out=rstd[:], in_=vtmp[:])
                    a_t = gnp.tile([P, 1], F32, tag="a_t")
                    nc.vector.tensor_mul(a_t[:], rstd[:], gam_sb[:, ct:ct + 1])
                    d_t = gnp.tile([P, 1], F32, tag="d_t")
                    nc.vector.tensor_mul(d_t[:], gstat[:, 0:1], a_t[:])
                    nc.vector.tensor_tensor(
                        out=d_t[:], in0=bet_sb[:, ct:ct + 1], in1=d_t[:],
                        op=mybir.AluOpType.subtract,
                    )
                    for s in range(8):
                        nc.scalar.activation(
                            out=hn_bf[ct][:, s * 512:(s + 1) * 512],
                            in_=x_sb[:, s * 512:(s + 1) * 512],
                            func=mybir.ActivationFunctionType.Identity,
                            scale=a_t[:], bias=d_t[:],
                        )


                # deferred weight loads (after x so groupnorm owns DMA at t=0)
                for t in range(CT):
                    nc.sync.dma_start(out=wq_sb[t][:], in_=wq_t[t])
                    nc.sync.dma_start(out=wk_sb[t][:], in_=wk_t[t])
                    nc.sync.dma_start(out=wv_sb[t][:], in_=wv_t[t])
                    nc.sync.dma_start(out=wo_sb[t][:], in_=wo_t[t])
                nc.sync.dma_start(
                    out=bv_bc[:],
                    in_=bass.AP(tensor=bv_h.tensor, offset=bv_h.offset, ap=[[0, P], [1, C]]),
                )

                # =========== Phase B: projections ===========
                # q[ct][c, i] (2048 query tokens), k[ct][c, j] (all 4096)
                for co in range(CT):
                    for isl in range(NSTRIP):
                        ps = pjp.tile([P, 512], F32, tag="pj")
                        for t in range(CT):
                            nc.tensor.matmul(
                                ps[:],
                                lhsT=wq_sb[t][:, co * P:(co + 1) * P],
                                rhs=hn_bf[t][:, isl * 512:(isl + 1) * 512],
                                start=(t == 0), stop=(t == CT - 1),
                            )
                        nc.vector.tensor_scalar_add(
                            out=q_bf[co][:, isl * 512:(isl + 1) * 512],
                            in0=ps[:], scalar1=bq_sb[:, co:co + 1],
                        )
                    for jsl in range(N // 512):
                        ps = pjp.tile([P, 512], F32, tag="pj")
                        for t in range(CT):
                            nc.tensor.matmul(
                                ps[:],
                                lhsT=wk_sb[t][:, co * P:(co + 1) * P],
                                rhs=hn_bf[t][:, jsl * 512:(jsl + 1) * 512],
                                start=(t == 0), stop=(t == CT - 1),
                            )
                        nc.vector.tensor_scalar_add(
                            out=k_bf[co][:, jsl * 512:(jsl + 1) * 512],
                            in0=ps[:], scalar1=bk_sb[:, co:co + 1],
                        )
                # v[jt][j, c] (token-major: one matmul per 128-token tile)
                for jt in range(JT):
                    ps = mmp.tile([P, 512], F32, tag="mm")
                    for t in range(CT):
                        nc.tensor.matmul(
                            ps[:],
                            lhsT=hn_bf[t][:, jt * P:(jt + 1) * P],
                            rhs=wv_sb[t][:],
                            start=(t == 0), stop=(t == CT - 1),
                        )
                    nc.vector.tensor_tensor(
                        out=v_bf[jt][:], in0=ps[:], in1=bv_bc[:],
                        op=mybir.AluOpType.add,
                    )


            with tc.tile_pool(name="hacc", bufs=4, space="PSUM") as hp, \
                 tc.tile_pool(name="lps", bufs=1, space="PSUM") as lp, \
                 tc.tile_pool(name="attn", bufs=1) as ap_, \
                 tc.tile_pool(name="lsb", bufs=2) as lsp, \
                 tc.tile_pool(name="xres", bufs=3) as xrp, \
                 tc.tile_pool(name="outt", bufs=3) as otp:

                # =========== Phase C: attention, software-pipelined strips ===========
                pT = [ap_.tile([P, 512], BF16, tag=f"pT{j}", name=f"pT{j}") for j in range(JT)]

                def emit_strip_core(st):
                    """scores -> exp -> colsums -> l roundtrip -> h matmuls -> h evac.
                    Returns the strip's h_bf tiles (normalized, bf16)."""
                    i0 = st * 512
                    for jt in range(JT):
                        ps = mmp.tile([P, 512], F32, tag="mm", name=f"s{st}_{jt}")
                        for t in range(CT):
                            nc.tensor.matmul(
                                ps[:],
                                lhsT=k_bf[t][:, jt * P:(jt + 1) * P],
                                rhs=q_bf[t][:, i0:i0 + 512],
                                start=(t == 0), stop=(t == CT - 1),
                            )
                        nc.scalar.activation(
                            out=pT[jt][:], in_=ps[:],
                            func=mybir.ActivationFunctionType.Exp,
                            scale=SCALE,
                        )
                    acc = lsp.tile([P, 512], F32, tag="lacc", name=f"lacc{st}")
                    nc.vector.tensor_tensor(
                        out=acc[:], in0=pT[0][:], in1=pT[1][:],
                        op=mybir.AluOpType.add,
                    )
                    for jt in range(2, JT):
                        nc.vector.tensor_tensor(
                            out=acc[:], in0=acc[:], in1=pT[jt][:],
                            op=mybir.AluOpType.add,
                        )
                    psl = lp.tile([1, 512], F32, tag="l", name=f"l{st}")
                    nc.tensor.matmul(
                        psl[:], lhsT=ones_f32[:], rhs=acc[:],
                        start=True, stop=True,
                    )
                    # 1/l on the single-partition row, then broadcast to all
                    # 128 partitions with a K=1 ones-matmul (all on-chip)
                    rl1 = lsp.tile([1, 512], F32, tag="rl1")
                    nc.vector.reciprocal(out=rl1[:], in_=psl[:])
                    psb = mmp.tile([P, 512], F32, tag="mm", name=f"rlbps{st}")
                    nc.tensor.matmul(psb[:], lhsT=ones1[:], rhs=rl1[:],
                                     start=True, stop=True)
                    rlb = lsp.tile([P, 512], F32, tag="rlb", name=f"rlb{st}")
                    nc.vector.tensor_copy(rlb[:], psb[:])
                    # h^T[c, i] = sum_j v[j, c] p[j, i] -- direct hT, no transposes
                    hps = [hp.tile([P, 512], F32, tag="h", name=f"hps{st}_{i}")
                           for i in range(CT)]
                    for jt in range(JT):
                        for cb in range(CT):
                            nc.tensor.matmul(
                                hps[cb][:],
                                lhsT=v_bf[jt][:, cb * P:(cb + 1) * P],
                                rhs=pT[jt][:],
                                start=(jt == 0), stop=(jt == JT - 1),
                            )
                    # normalize + evacuate straight into hT (bf16)
                    for cb in range(CT):
                        nc.vector.tensor_mul(
                            hT_bf[cb][:, i0:i0 + 512], hps[cb][:], rlb[:]
                        )
                    return None

                def emit_strip_tail(st, h_bfs):
                    """output projection + residual for one strip."""
                    i0 = st * 512
                    for co in range(CT):
                        ps = mmp.tile([P, 512], F32, tag="mm",
                                      name=f"op{st}_{co}")
                        for t in range(CT):
                            nc.tensor.matmul(
                                ps[:],
                                lhsT=wo_sb[t][:, co * P:(co + 1) * P],
                                rhs=hT_bf[t][:, i0:i0 + 512],
                                start=(t == 0), stop=(t == CT - 1),
                            )
                        xr = xrp.tile([P, 512], F32, tag="xr")
                        nc.sync.dma_start(
                            out=xr[:], in_=x_t[co][:, i0:i0 + 512]
                        )
                        ot = otp.tile([P, 512], F32, tag="ot")
                        nc.vector.tensor_scalar_add(
                            out=ot[:], in0=ps[:], scalar1=bo_sb[:, co:co + 1]
                        )
                        nc.vector.tensor_tensor(
                            out=ot[:], in0=ot[:], in1=xr[:],
                            op=mybir.AluOpType.add,
                        )
                        nc.sync.dma_start(
                            out=out_t[co][:, i0:i0 + 512], in_=ot[:]
                        )

                prev = None
                for st in range(NSTRIP):
                    h_bfs = emit_strip_core(st)
                    if prev is not None:
                        emit_strip_tail(prev[0], prev[1])
                    prev = (st, h_bfs)
                emit_strip_tail(prev[0], prev[1])

    nc.finalize()
    return nc


def kernel(**inputs):
    if "nc" not in _CACHE:
        _CACHE["nc"] = build_bass()
    nc = _CACHE["nc"]

    x = np.ascontiguousarray(np.asarray(inputs["x"], dtype=np.float32))
    B = x.shape[0]
    xf = x.reshape(B, C, N)

    def bfT(w):
        return np.ascontiguousarray(
            np.asarray(w, dtype=np.float32).T.astype(ml_dtypes.bfloat16)
        )

    shared = {
        "wqT": bfT(inputs["wq"]), "wkT": bfT(inputs["wk"]),
        "wvT": bfT(inputs["wv"]), "woT": bfT(inputs["wo"]),
        "bq": np.ascontiguousarray(np.asarray(inputs["bq"], np.float32)),
        "bk": np.ascontiguousarray(np.asarray(inputs["bk"], np.float32)),
        "bv": np.ascontiguousarray(np.asarray(inputs["bv"], np.float32)),
        "bo": np.ascontiguousarray(np.asarray(inputs["bo"], np.float32)),
        "gam": np.ascontiguousarray(np.asarray(inputs["norm_g"], np.float32)),
        "bet": np.ascontiguousarray(np.asarray(inputs["norm_b"], np.float32)),
    }

    in_maps = []
    for core in range(2 * B):
        b, half = core // 2, core % 2
        xb = xf[b]
        if half:
            xb = np.concatenate([xb[:, NQ:], xb[:, :NQ]], axis=1)
        in_maps.append({"x": np.ascontiguousarray(xb), **shared})

    import os
    trace = bool(os.environ.get("BASS_KERNEL_TRACE"))
    res = run_bass_kernel_spmd(
        nc, in_maps, core_ids=list(range(2 * B)), trace=trace,
        trace_cores=list(range(2 * B)) if trace else None,
    )
    _CACHE["last_results"] = res

    out = np.empty((B, C, N), np.float32)
    for core in range(2 * B):
        b, half = core // 2, core % 2
        out[b][:, half * NQ:(half + 1) * NQ] = res.results[core]["out"]
    return out.reshape(B, C, 64, 64)



# revision 21
# speedup vs baseline: 2.0281x; 2.0281x over previous
"""Trainium2 Bass kernel for nn_AttnBlock (GroupNorm + single-head 4096-token
attention + residual), sharded over 8 NeuronCores.

Sharding: data-parallel over batch B=4, sequence-parallel x2 over the 4096
query tokens -> 8 shards. Each core computes k/v for its full batch
(duplicated across the 2 token-halves) and q/attention/out-proj for its 2048
query tokens. The token axis is rolled on the host for the second half so a
single SPMD NEFF serves all cores (softmax over keys is order-invariant,
groupnorm stats are token-permutation-invariant).

v2: all large matmuls run in fp8(e4m3) with MatmulPerfMode.DoubleRow
(K=256 per instruction, 0.5 cycles/row). Attention output is kept
UNNORMALIZED through the v-matmul (exp biased by EXP_BIAS so unnormalized
h stays inside fp8 range); the softmax denominator l (computed by an M=1
ones-matmul on the PE over the quantized p tiles) is applied per-query to
the *out-projection* PSUM instead, which removes the l -> h-evac dependency.
x stays resident in SBUF for the residual (no reload). PSUM->SBUF
evacuations are spread across ACT/DVE/Pool.

Self-contained: hardcodes all shapes; only needs the concourse runtime.
"""

import numpy as np
import ml_dtypes

import concourse.bass as bass
import concourse.bacc as bacc
import concourse.tile as tile
from concourse import mybir
from concourse.bass_utils import run_bass_kernel_spmd

P = 128                 # partitions
C = 512                 # channels
N = 4096                # tokens (64*64)
NQ = 2048               # query tokens per core
CT = C // P             # 4 channel tiles of 128
CP = 2                  # channel pair-tiles (DoubleRow K=256)
JT = N // P             # 32 key-token tiles of 128
JP = JT // 2            # 16 key-token pair-tiles
NSTRIP = NQ // 512      # 4 query strips of 512
GS = 16                 # channels per group
NG = P // GS            # 8 groups per channel tile
EPS = 1e-6
SCALE = float(C) ** -0.5
EXP_BIAS = -2.5         # keeps unnormalized h inside fp8-e4m3 range (240)
V_SCALE = 0.125         # v stored as v/8 in fp8; wo scaled x8 on the host
F32 = mybir.dt.float32
BF16 = mybir.dt.bfloat16
F8 = mybir.dt.float8e4
DR = mybir.MatmulPerfMode.DoubleRow
ADD = mybir.AluOpType.add
MULT = mybir.AluOpType.mult
IDENT = mybir.ActivationFunctionType.Identity
EXP = mybir.ActivationFunctionType.Exp

_CACHE = {}


def build_bass(debug=False):
    nc = bacc.Bacc(None, target_bir_lowering=False)

    x_h = nc.dram_tensor("x", [C, N], F32, kind="ExternalInput")[:]
    wq_h = nc.dram_tensor("wqT", [C, C], F8, kind="ExternalInput")[:]
    wk_h = nc.dram_tensor("wkT", [C, C], F8, kind="ExternalInput")[:]
    wv_h = nc.dram_tensor("wvT", [C, C], F8, kind="ExternalInput")[:]
    wo_h = nc.dram_tensor("woT", [C, C], F8, kind="ExternalInput")[:]
    bq_h = nc.dram_tensor("bq", [C], F32, kind="ExternalInput")[:]
    bk_h = nc.dram_tensor("bk", [C], F32, kind="ExternalInput")[:]
    bv_h = nc.dram_tensor("bv", [C], F32, kind="ExternalInput")[:]
    bo_h = nc.dram_tensor("bo", [C], F32, kind="ExternalInput")[:]
    gam_h = nc.dram_tensor("gam", [C], F32, kind="ExternalInput")[:]
    bet_h = nc.dram_tensor("bet", [C], F32, kind="ExternalInput")[:]
    out_h = nc.dram_tensor("out", [C, NQ], F32, kind="ExternalOutput")[:]

    dbg = {}
    if debug:
        dbg["hn"] = nc.dram_tensor("d_hn", [CP, P, 2, N], F8, kind="ExternalOutput")[:]
        dbg["q"] = nc.dram_tensor("d_q", [CP, P, 2, NQ], F8, kind="ExternalOutput")[:]
        dbg["k"] = nc.dram_tensor("d_k", [CP, P, 2, N], F8, kind="ExternalOutput")[:]
        dbg["v"] = nc.dram_tensor("d_v", [JP, P, 2, C], F8, kind="ExternalOutput")[:]
        dbg["pT"] = nc.dram_tensor("d_pT", [JP, P, 2, 512], F8, kind="ExternalOutput")[:]
        dbg["hT"] = nc.dram_tensor("d_hT", [CP, P, 2, NQ], F8, kind="ExternalOutput")[:]

    g8_np = np.zeros((P, NG), np.float32)
    g8T_np = np.zeros((NG, P), np.float32)
    for c in range(P):
        g8_np[c, c // GS] = 1.0 / GS
        g8T_np[c // GS, c] = 1.0
    g8_h = nc.inline_tensor(g8_np, name="g8")[:]
    g8T_h = nc.inline_tensor(g8T_np, name="g8T")[:]

    x_t = x_h.rearrange("(t p) n -> t p n", p=P)          # [4,128,4096]
    out_t = out_h.rearrange("(t p) n -> t p n", p=P)      # [4,128,2048]

    def col4(ap1d):
        # [512] dram vector -> [128,4] sbuf layout (column ct holds chans ct*128..)
        return bass.AP(tensor=ap1d.tensor, offset=ap1d.offset, ap=[[1, P], [P, CT]])

    with tile.TileContext(nc) as tc:
        with tc.tile_pool(name="consts", bufs=1) as cp, \
             tc.tile_pool(name="wgt", bufs=1) as wp, \
             tc.tile_pool(name="xres", bufs=1) as xp, \
             tc.tile_pool(name="qkv", bufs=1) as qkvp, \
             tc.tile_pool(name="hT", bufs=1) as hTp:

            # ---- constants ----
            eps_t = cp.tile([P, 1], F32, tag="eps")
            nc.vector.memset(eps_t[:], EPS)
            ebias_t = cp.tile([P, 1], F32, tag="ebias")
            nc.vector.memset(ebias_t[:], EXP_BIAS)
            # DoubleRow ldweights needs the k-pair dim step to be a multiple
            # of 16 bytes, so pad the ones column out to 16
            ones_f8 = cp.tile([P, 2, 16], F8, tag="ones8")
            nc.vector.memset(ones_f8[:], 1.0)
            g8_sb = cp.tile([P, NG], F32, tag="g8")
            nc.sync.dma_start(out=g8_sb[:], in_=g8_h)
            g8T_sb = cp.tile([NG, P], F32, tag="g8T")
            nc.sync.dma_start(out=g8T_sb[:], in_=g8T_h)
            bq_sb = cp.tile([P, CT], F32, tag="bq")
            nc.sync.dma_start(out=bq_sb[:], in_=col4(bq_h))
            bk_sb = cp.tile([P, CT], F32, tag="bk")
            nc.sync.dma_start(out=bk_sb[:], in_=col4(bk_h))
            bo_sb = cp.tile([P, CT], F32, tag="bo")
            nc.sync.dma_start(out=bo_sb[:], in_=col4(bo_h))
            gam_sb = cp.tile([P, CT], F32, tag="gam")
            nc.sync.dma_start(out=gam_sb[:], in_=col4(gam_h))
            bet_sb = cp.tile([P, CT], F32, tag="bet")
            nc.sync.dma_start(out=bet_sb[:], in_=col4(bet_h))
            bv_bc = cp.tile([P, C], F32, tag="bvbc")

            # ---- persistent activations (fp8, DoubleRow pair layout) ----
            # x kept resident for the residual add in the tail.
            x_sb = [xp.tile([P, N], F32, tag=f"x{t}", name=f"x{t}")
                    for t in range(CT)]
            hn_f8 = [qkvp.tile([P, 2, N], F8, tag=f"hn{t}", name=f"hn{t}")
                     for t in range(CP)]
            q_f8 = [qkvp.tile([P, 2, NQ], F8, tag=f"q{t}", name=f"q{t}")
                    for t in range(CP)]
            k_f8 = [qkvp.tile([P, 2, N], F8, tag=f"k{t}", name=f"k{t}")
                    for t in range(CP)]
            v_f8 = [qkvp.tile([P, 2, C], F8, tag=f"v{j}", name=f"v{j}")
                    for j in range(JP)]
            hT_f8 = [hTp.tile([P, 2, NQ], F8, tag=f"hT{t}", name=f"hT{t}")
                     for t in range(CP)]
            w_sb = {}
            for wname in ("wq", "wk", "wv", "wo"):
                w_sb[wname] = [wp.tile([P, 2, C], F8, tag=f"{wname}{t}",
                                       name=f"{wname}{t}") for t in range(CP)]

            # =========== Phase A: groupnorm -> hn (fp8) ===========
            with tc.tile_pool(name="gnsb", bufs=2) as gnp, \
                 tc.tile_pool(name="gnps", bufs=2, space="PSUM") as gnps:

                for ct in range(CT):
                    # chunked load + per-chunk stats so stats start on the
                    # first chunk instead of after the full 2MB tile
                    stats = gnp.tile([P, 8, 6], F32, tag="stats")
                    for s in range(8):
                        nc.sync.dma_start(
                            out=x_sb[ct][:, s * 512:(s + 1) * 512],
                            in_=x_t[ct][:, s * 512:(s + 1) * 512],
                        )
                        nc.vector.bn_stats(
                            out=stats[:, s, :], in_=x_sb[ct][:, s * 512:(s + 1) * 512]
                        )
                    mv = gnp.tile([P, 2], F32, tag="mv")
                    nc.vector.bn_aggr(out=mv[:], in_=stats[:])
                    # cstat = [mean, E[x^2]] per channel
                    cstat = gnp.tile([P, 2], F32, tag="cstat")
                    nc.vector.tensor_copy(cstat[:, 0:1], mv[:, 0:1])
                    nc.vector.tensor_mul(cstat[:, 1:2], mv[:, 0:1], mv[:, 0:1])
                    nc.vector.tensor_add(cstat[:, 1:2], cstat[:, 1:2], mv[:, 1:2])
                    # group-average then broadcast back to channels (PE)
                    psA = gnps.tile([NG, 2], F32, tag="gn")
                    nc.tensor.matmul(psA[:], lhsT=g8_sb[:], rhs=cstat[:],
                                     start=True, stop=True)
                    gt = gnp.tile([NG, 2], F32, tag="gt")
                    nc.vector.tensor_copy(gt[:], psA[:])
                    psB = gnps.tile([P, 2], F32, tag="gn")
                    nc.tensor.matmul(psB[:], lhsT=g8T_sb[:], rhs=gt[:],
                                     start=True, stop=True)
                    gstat = gnp.tile([P, 2], F32, tag="gstat")
                    nc.vector.tensor_copy(gstat[:], psB[:])
                    # a = gamma * rsqrt(gvar+eps); d = beta - gmean * a
                    vtmp = gnp.tile([P, 1], F32, tag="vtmp")
                    nc.vector.tensor_mul(vtmp[:], gstat[:, 0:1], gstat[:, 0:1])
                    nc.vector.tensor_tensor(
                        out=vtmp[:], in0=gstat[:, 1:2], in1=vtmp[:],
                        op=mybir.AluOpType.subtract,
                    )
                    nc.scalar.activation(
                        out=vtmp[:], in_=vtmp[:],
                        func=mybir.ActivationFunctionType.Sqrt,
                        bias=eps_t[:], scale=1.0,
                    )
                    rstd = gnp.tile([P, 1], F32, tag="rstd")
                    nc.vector.reciprocal(out=rstd[:], in_=vtmp[:])
                    a_t = gnp.tile([P, 1], F32, tag="a_t")
                    nc.vector.tensor_mul(a_t[:], rstd[:], gam_sb[:, ct:ct + 1])
                    d_t = gnp.tile([P, 1], F32, tag="d_t")
                    nc.vector.tensor_mul(d_t[:], gstat[:, 0:1], a_t[:])
                    nc.vector.tensor_tensor(
                        out=d_t[:], in0=bet_sb[:, ct:ct + 1], in1=d_t[:],
                        op=mybir.AluOpType.subtract,
                    )
                    # apply: split between ACT and DVE so the last tile's
                    # apply doesn't serialize behind one engine
                    hdst = hn_f8[ct // 2]
                    nc.scalar.activation(
                        out=hdst[:, ct % 2, 0:2048],
                        in_=x_sb[ct][:, 0:2048],
                        func=IDENT, scale=a_t[:], bias=d_t[:],
                    )
                    nc.vector.tensor_scalar(
                        out=hdst[:, ct % 2, 2048:4096],
                        in0=x_sb[ct][:, 2048:4096],
                        scalar1=a_t[:], scalar2=d_t[:],
                        op0=MULT, op1=ADD,
                    )

            # deferred weight loads (after x so groupnorm owns DMA at t=0)
            wq_t = wq_h.rearrange("(t p) o -> t p o", p=P)
            wk_t = wk_h.rearrange("(t p) o -> t p o", p=P)
            wv_t = wv_h.rearrange("(t p) o -> t p o", p=P)
            wo_t = wo_h.rearrange("(t p) o -> t p o", p=P)
            for t in range(CP):
                for s in range(2):
                    nc.sync.dma_start(out=w_sb["wq"][t][:, s, :], in_=wq_t[2 * t + s])
                    nc.sync.dma_start(out=w_sb["wk"][t][:, s, :], in_=wk_t[2 * t + s])
                    nc.sync.dma_start(out=w_sb["wv"][t][:, s, :], in_=wv_t[2 * t + s])
                    nc.sync.dma_start(out=w_sb["wo"][t][:, s, :], in_=wo_t[2 * t + s])
            nc.sync.dma_start(
                out=bv_bc[:],
                in_=bass.AP(tensor=bv_h.tensor, offset=bv_h.offset, ap=[[0, P], [1, C]]),
            )
            # pre-scaled bias for the v/8 layout
            bv_bc8 = cp.tile([P, C], F32, tag="bvbc8")
            nc.vector.tensor_scalar_mul(out=bv_bc8[:], in0=bv_bc[:],
                                        scalar1=V_SCALE)
            # bo replicated along the free dim so the tail's bias add can be
            # a plain tensor_tensor on Pool (no TensorScalar there)
            zrow = cp.tile([P, 512], F32, tag="zrow")
            nc.vector.memset(zrow[:], 0.0)
            bo_rep = [cp.tile([P, 512], F32, tag=f"borep{co}",
                              name=f"borep{co}") for co in range(CT)]
            for co in range(CT):
                nc.vector.tensor_scalar_add(out=bo_rep[co][:], in0=zrow[:],
                                            scalar1=bo_sb[:, co:co + 1])

            # =========== Phase B: projections (fp8 DoubleRow) ===========
            with tc.tile_pool(name="projps", bufs=3, space="PSUM") as pjp:

                def evac_bias(idx, out, ps, bcol):
                    # alternate psum->sbuf evacuation between ACT and DVE
                    # (GPSIMD cannot read PSUM)
                    if idx % 2 == 0:
                        nc.scalar.activation(out=out, in_=ps, func=IDENT,
                                             bias=bcol, scale=1.0)
                    else:
                        nc.vector.tensor_scalar_add(out=out, in0=ps, scalar1=bcol)

                # k[c_out, j] for all 4096 keys; evac order (jsl, co) so the
                # first score tiles unblock as early as possible
                ei = 0
                for jsl in range(N // 512):
                    for co in range(CT):
                        ps = pjp.tile([P, 512], F32, tag="pj")
                        for t in range(CP):
                            nc.tensor.matmul(
                                ps[:],
                                lhsT=w_sb["wk"][t][:, :, co * P:(co + 1) * P],
                                rhs=hn_f8[t][:, :, jsl * 512:(jsl + 1) * 512],
                                start=(t == 0), stop=(t == CP - 1),
                                perf_mode=DR,
                            )
                        evac_bias(ei, k_f8[co // 2][:, co % 2, jsl * 512:(jsl + 1) * 512],
                                  ps[:], bk_sb[:, co:co + 1])
                        ei += 1
                # q[c_out, i] for this core's 2048 query tokens
                for isl in range(NSTRIP):
                    for co in range(CT):
                        ps = pjp.tile([P, 512], F32, tag="pj")
                        for t in range(CP):
                            nc.tensor.matmul(
                                ps[:],
                                lhsT=w_sb["wq"][t][:, :, co * P:(co + 1) * P],
                                rhs=hn_f8[t][:, :, isl * 512:(isl + 1) * 512],
                                start=(t == 0), stop=(t == CP - 1),
                                perf_mode=DR,
                            )
                        evac_bias(ei, q_f8[co // 2][:, co % 2, isl * 512:(isl + 1) * 512],
                                  ps[:], bq_sb[:, co:co + 1])
                        ei += 1
                # v[j, c_out] token-major (one matmul pair per 128-token tile)
                for jt in range(JT):
                    ps = pjp.tile([P, 512], F32, tag="pj")
                    for t in range(CP):
                        nc.tensor.matmul(
                            ps[:],
                            lhsT=hn_f8[t][:, :, jt * P:(jt + 1) * P],
                            rhs=w_sb["wv"][t][:],
                            start=(t == 0), stop=(t == CP - 1),
                            perf_mode=DR,
                        )
                    nc.vector.scalar_tensor_tensor(
                        out=v_f8[jt // 2][:, jt % 2, :], in0=ps[:],
                        scalar=V_SCALE, in1=bv_bc8[:], op0=MULT, op1=ADD,
                    )

            # =========== Phase C: attention ===========
            with tc.tile_pool(name="scps", bufs=3, space="PSUM") as scp, \
                 tc.tile_pool(name="hacc", bufs=4, space="PSUM") as hp, \
                 tc.tile_pool(name="lps", bufs=1, space="PSUM") as lp, \
                 tc.tile_pool(name="attn", bufs=1) as ap_, \
                 tc.tile_pool(name="lsb", bufs=2) as lsp, \
                 tc.tile_pool(name="t1p", bufs=3) as t1p, \
                 tc.tile_pool(name="outt", bufs=3) as otp:

                pT = [ap_.tile([P, 2, 512], F8, tag=f"pT{j}", name=f"pT{j}")
                      for j in range(JP)]

                for st in range(NSTRIP):
                    i0 = st * 512
                    hps = [hp.tile([P, 512], F32, tag="h", name=f"hps{st}_{i}")
                           for i in range(CT)]
                    lt = lp.tile([1, 512], F32, tag="l", name=f"l{st}")

                    def emit_h_l(jp):
                        for cb in range(CT):
                            nc.tensor.matmul(
                                hps[cb][:],
                                lhsT=v_f8[jp][:, :, cb * P:(cb + 1) * P],
                                rhs=pT[jp][:],
                                start=(jp == 0), stop=(jp == JP - 1),
                                perf_mode=DR,
                            )
                        nc.tensor.matmul(
                            lt[:], lhsT=ones_f8[:, :, 0:1], rhs=pT[jp][:],
                            start=(jp == 0), stop=(jp == JP - 1),
                            perf_mode=DR,
                        )

                    # scores + exp, with h/l matmuls interleaved one pair
                    # behind so the PE fills its exp-wait stalls
                    for jt in range(JT):
                        sc = scp.tile([P, 512], F32, tag="sc",
                                      name=f"s{st}_{jt}")
                        for t in range(CP):
                            nc.tensor.matmul(
                                sc[:],
                                lhsT=k_f8[t][:, :, jt * P:(jt + 1) * P],
                                rhs=q_f8[t][:, :, i0:i0 + 512],
                                start=(t == 0), stop=(t == CP - 1),
                                perf_mode=DR,
                            )
                        nc.scalar.activation(
                            out=pT[jt // 2][:, jt % 2, :], in_=sc[:],
                            func=EXP, scale=SCALE, bias=ebias_t[:],
                        )
                        if jt % 2 == 1 and jt >= 3:
                            emit_h_l((jt - 1) // 2 - 1)
                    emit_h_l(JP - 1)

                    # 1/l on partition 0, broadcast to all partitions on Pool
                    if debug:
                        if st == 0:
                            for jp in range(JP):
                                nc.sync.dma_start(out=dbg["pT"][jp], in_=pT[jp][:])
                    rl1 = lsp.tile([1, 512], F32, tag="rl1", name=f"rl1{st}")
                    nc.vector.reciprocal(out=rl1[:], in_=lt[:])
                    rlb = lsp.tile([P, 512], F32, tag="rlb", name=f"rlb{st}")
                    nc.gpsimd.partition_broadcast(rlb[:], rl1[:])

                    # evacuate unnormalized h straight to fp8 (no l dep)
                    for cb in range(CT):
                        if cb % 2 == 0:
                            nc.vector.tensor_copy(
                                hT_f8[cb // 2][:, cb % 2, i0:i0 + 512], hps[cb][:]
                            )
                        else:
                            nc.scalar.activation(
                                out=hT_f8[cb // 2][:, cb % 2, i0:i0 + 512],
                                in_=hps[cb][:], func=IDENT,
                            )

                    # out-projection; normalize by 1/l here, add bias+residual
                    for co in range(CT):
                        ps = hp.tile([P, 512], F32, tag="h", name=f"op{st}_{co}")
                        for t in range(CP):
                            nc.tensor.matmul(
                                ps[:],
                                lhsT=w_sb["wo"][t][:, :, co * P:(co + 1) * P],
                                rhs=hT_f8[t][:, :, i0:i0 + 512],
                                start=(t == 0), stop=(t == CP - 1),
                                perf_mode=DR,
                            )
                        t1 = t1p.tile([P, 512], F32, tag="t1")
                        nc.vector.tensor_tensor(out=t1[:], in0=ps[:],
                                                in1=rlb[:], op=MULT)
                        # final adds on Pool: all-SBUF tensor_tensor, frees
                        # ACT/DVE for psum work
                        u = t1p.tile([P, 512], F32, tag="u")
                        nc.gpsimd.tensor_tensor(out=u[:], in0=t1[:],
                                                in1=bo_rep[co][:], op=ADD)
                        ot = otp.tile([P, 512], F32, tag="ot")
                        nc.gpsimd.tensor_tensor(out=ot[:], in0=u[:],
                                                in1=x_sb[co][:, i0:i0 + 512],
                                                op=ADD)
                        nc.sync.dma_start(
                            out=out_t[co][:, i0:i0 + 512], in_=ot[:]
                        )

            if debug:
                for t in range(CP):
                    nc.sync.dma_start(out=dbg["hn"][t], in_=hn_f8[t][:])
                    nc.sync.dma_start(out=dbg["q"][t], in_=q_f8[t][:])
                    nc.sync.dma_start(out=dbg["k"][t], in_=k_f8[t][:])
                    nc.sync.dma_start(out=dbg["hT"][t], in_=hT_f8[t][:])
                for jp in range(JP):
                    nc.sync.dma_start(out=dbg["v"][jp], in_=v_f8[jp][:])

    nc.finalize()
    return nc


def kernel(**inputs):
    if "nc" not in _CACHE:
        _CACHE["nc"] = build_bass()
    nc = _CACHE["nc"]

    x = np.ascontiguousarray(np.asarray(inputs["x"], dtype=np.float32))
    B = x.shape[0]
    xf = x.reshape(B, C, N)

    def f8T(w, scale=1.0):
        return np.ascontiguousarray(
            (np.asarray(w, dtype=np.float32).T * scale).astype(
                ml_dtypes.float8_e4m3)
        )

    shared = {
        "wqT": f8T(inputs["wq"]), "wkT": f8T(inputs["wk"]),
        "wvT": f8T(inputs["wv"]), "woT": f8T(inputs["wo"], 1.0 / V_SCALE),
        "bq": np.ascontiguousarray(np.asarray(inputs["bq"], np.float32)),
        "bk": np.ascontiguousarray(np.asarray(inputs["bk"], np.float32)),
        "bv": np.ascontiguousarray(np.asarray(inputs["bv"], np.float32)),
        "bo": np.ascontiguousarray(np.asarray(inputs["bo"], np.float32)),
        "gam": np.ascontiguousarray(np.asarray(inputs["norm_g"], np.float32)),
        "bet": np.ascontiguousarray(np.asarray(inputs["norm_b"], np.float32)),
    }

    in_maps = []
    for core in range(2 * B):
        b, half = core // 2, core % 2
        xb = xf[b]
        if half:
            xb = np.concatenate([xb[:, NQ:], xb[:, :NQ]], axis=1)
        in_maps.append({"x": np.ascontiguousarray(xb), **shared})

    import os
    trace = bool(os.environ.get("BASS_KERNEL_TRACE"))
    res = run_bass_kernel_spmd(
        nc, in_maps, core_ids=list(range(2 * B)), trace=trace,
        trace_cores=list(range(2 * B)) if trace else None,
    )
    _CACHE["last_results"] = res

    out = np.empty((B, C, N), np.float32)
    for core in range(2 * B):
        b, half = core // 2, core % 2
        out[b][:, half * NQ:(half + 1) * NQ] = res.results[core]["out"]
    return out.reshape(B, C, 64, 64)


# revision 43
# speedup vs baseline: 2.6929x; 1.3278x over previous
"""Trainium2 Bass kernel for nn_AttnBlock (GroupNorm + single-head 4096-token
attention + residual), sharded over 8 NeuronCores.

Sharding: data-parallel over batch B=4, sequence-parallel x2 over the 4096
query tokens -> 8 shards. Each core computes k/v for its full batch
(duplicated across the 2 token-halves) and q/attention/out-proj for its 2048
query tokens. The token axis is rolled on the host for the second half so a
single SPMD NEFF serves all cores (softmax over keys is order-invariant,
groupnorm stats are token-permutation-invariant).

v3 pipeline: all large matmuls are fp8(e4m3) MatmulPerfMode.DoubleRow
(K=256/instr). The ACT engine's exp stream is the spine: pT (exp scores) is
double-buffered across strips so exps never wait on downstream consumers.
Strip st's h/l matmuls run inside strip st+1's score window; the v
projection hides inside strip 0's score window. The softmax denominator l
(M=1 ones-matmul over the quantized pT tiles) normalizes h at evacuation.
The v bias is folded into bo on the host (softmax weights sum to 1).
x stays resident in SBUF for the residual. PSUM->SBUF evacuations use
per-engine psum pools so ACT and DVE drain in parallel.

Self-contained: hardcodes all shapes; only needs the concourse runtime.
"""

import numpy as np
import ml_dtypes

import concourse.bass as bass
import concourse.bacc as bacc
import concourse.tile as tile
from concourse import mybir
from concourse.bass_utils import run_bass_kernel_spmd

P = 128                 # partitions
C = 512                 # channels
N = 4096                # tokens (64*64)
NQ = 2048               # query tokens per core
CT = C // P             # 4 channel tiles of 128
CP = 2                  # channel pair-tiles (DoubleRow K=256)
JT = N // P             # 32 key-token tiles of 128
JP = JT // 2            # 16 key-token pair-tiles
NSTRIP = NQ // 512      # 4 query strips of 512
GS = 16                 # channels per group
NG = P // GS            # 8 groups per channel tile
EPS = 1e-6
SCALE = float(C) ** -0.5
EXP_BIAS = -2.5         # keeps unnormalized h inside fp8-e4m3 range (240)
V_SCALE = 0.125         # v stored as v/8 in fp8; wo scaled x8 on the host
F32 = mybir.dt.float32
BF16 = mybir.dt.bfloat16
F8 = mybir.dt.float8e4
DR = mybir.MatmulPerfMode.DoubleRow
ADD = mybir.AluOpType.add
MULT = mybir.AluOpType.mult
IDENT = mybir.ActivationFunctionType.Identity
EXP = mybir.ActivationFunctionType.Exp

_CACHE = {}


def build_bass(debug=False):
    nc = bacc.Bacc(None, target_bir_lowering=False)

    x_h = nc.dram_tensor("x", [C, N], F32, kind="ExternalInput")[:]
    wq_h = nc.dram_tensor("wqT", [C, C], F8, kind="ExternalInput")[:]
    wk_h = nc.dram_tensor("wkT", [C, C], F8, kind="ExternalInput")[:]
    wv_h = nc.dram_tensor("wvT", [C, C], F8, kind="ExternalInput")[:]
    wo_h = nc.dram_tensor("woT", [C, C], F8, kind="ExternalInput")[:]
    # all per-channel vectors pre-shaped on the host into one [128, 28]
    # tensor (col-major channel blocks): one contiguous DMA instead of five
    # 512-descriptor gathers. cols: bq bk bo gam bet (4 each), g8 (8)
    cvec_h = nc.dram_tensor("cvec", [P, 28], F32, kind="ExternalInput")[:]
    out_h = nc.dram_tensor("out", [C, NQ], F32, kind="ExternalOutput")[:]

    dbg = {}
    if debug:
        dbg["hn"] = nc.dram_tensor("d_hn", [CP, P, 2, N], F8, kind="ExternalOutput")[:]
        dbg["q"] = nc.dram_tensor("d_q", [CP, P, 2, NQ], F8, kind="ExternalOutput")[:]
        dbg["k"] = nc.dram_tensor("d_k", [CP, P, 2, N], F8, kind="ExternalOutput")[:]
        dbg["v"] = nc.dram_tensor("d_v", [JP, P, 2, C], F8, kind="ExternalOutput")[:]
        dbg["hT"] = nc.dram_tensor("d_hT", [CP, P, 2, NQ], F8, kind="ExternalOutput")[:]

    g8T_np = np.zeros((NG, P), np.float32)
    for c in range(P):
        g8T_np[c // GS, c] = 1.0
    g8T_h = nc.inline_tensor(g8T_np, name="g8T")[:]

    x_t = x_h.rearrange("(t p) n -> t p n", p=P)          # [4,128,4096]
    out_t = out_h.rearrange("(t p) n -> t p n", p=P)      # [4,128,2048]

    with tile.TileContext(nc) as tc:
        with tc.tile_pool(name="consts", bufs=1) as cp, \
             tc.tile_pool(name="wgt", bufs=1) as wp, \
             tc.tile_pool(name="xres", bufs=1) as xp, \
             tc.tile_pool(name="qkv", bufs=1) as qkvp, \
             tc.tile_pool(name="hT", bufs=1) as hTp:

            # ---- constants ----
            eps_t = cp.tile([P, 1], F32, tag="eps")
            nc.vector.memset(eps_t[:], EPS)
            ebias_t = cp.tile([P, 1], F32, tag="ebias")
            nc.vector.memset(ebias_t[:], EXP_BIAS)
            # DoubleRow ldweights needs the k-pair dim step to be a multiple
            # of 16 bytes, so pad the ones column out to 16
            ones_f8 = cp.tile([P, 2, 16], F8, tag="ones8")
            nc.vector.memset(ones_f8[:], 1.0)
            cvec_sb = cp.tile([P, 28], F32, tag="cvec")
            g8T_sb = cp.tile([NG, P], F32, tag="g8T")

            # ---- persistent activations (fp8, DoubleRow pair layout) ----
            x_sb = [xp.tile([P, N], F32, tag=f"x{t}", name=f"x{t}")
                    for t in range(CT)]
            hn_f8 = [qkvp.tile([P, 2, N], F8, tag=f"hn{t}", name=f"hn{t}")
                     for t in range(CP)]
            q_f8 = [qkvp.tile([P, 2, NQ], F8, tag=f"q{t}", name=f"q{t}")
                    for t in range(CP)]
            k_f8 = [qkvp.tile([P, 2, N], F8, tag=f"k{t}", name=f"k{t}")
                    for t in range(CP)]
            v_f8 = [qkvp.tile([P, 2, C], F8, tag=f"v{j}", name=f"v{j}")
                    for j in range(JP)]
            hT_f8 = [hTp.tile([P, 2, NQ], F8, tag=f"hT{t}", name=f"hT{t}")
                     for t in range(CP)]
            w_sb = {}
            for wname in ("wq", "wk", "wv", "wo"):
                w_sb[wname] = [wp.tile([P, 2, C], F8, tag=f"{wname}{t}",
                                       name=f"{wname}{t}") for t in range(CP)]

            # =========== Phase A: groupnorm -> hn (fp8) ===========
            with tc.tile_pool(name="gnsb", bufs=2) as gnp, \
                 tc.tile_pool(name="gnps", bufs=2, space="PSUM") as gnps:

                for ct in range(CT):
                    stats = gnp.tile([P, 8, 6], F32, tag="stats")
                    for s in range(8):
                        nc.sync.dma_start(
                            out=x_sb[ct][:, s * 512:(s + 1) * 512],
                            in_=x_t[ct][:, s * 512:(s + 1) * 512],
                        )
                        nc.vector.bn_stats(
                            out=stats[:, s, :], in_=x_sb[ct][:, s * 512:(s + 1) * 512]
                        )
                    if ct == 0:
                        # consts ride the DMA queue behind ct0's x chunks
                        nc.sync.dma_start(out=cvec_sb[:], in_=cvec_h)
                        nc.sync.dma_start(out=g8T_sb[:], in_=g8T_h)
                    mv = gnp.tile([P, 2], F32, tag="mv")
                    nc.vector.bn_aggr(out=mv[:], in_=stats[:])
                    cstat = gnp.tile([P, 2], F32, tag="cstat")
                    nc.vector.tensor_copy(cstat[:, 0:1], mv[:, 0:1])
                    nc.vector.tensor_mul(cstat[:, 1:2], mv[:, 0:1], mv[:, 0:1])
                    nc.vector.tensor_add(cstat[:, 1:2], cstat[:, 1:2], mv[:, 1:2])
                    psA = gnps.tile([NG, 2], F32, tag="gn")
                    nc.tensor.matmul(psA[:], lhsT=cvec_sb[:, 20:28], rhs=cstat[:],
                                     start=True, stop=True)
                    gt = gnp.tile([NG, 2], F32, tag="gt")
                    nc.vector.tensor_copy(gt[:], psA[:])
                    psB = gnps.tile([P, 2], F32, tag="gn")
                    nc.tensor.matmul(psB[:], lhsT=g8T_sb[:], rhs=gt[:],
                                     start=True, stop=True)
                    gstat = gnp.tile([P, 2], F32, tag="gstat")
                    nc.vector.tensor_copy(gstat[:], psB[:])
                    vtmp = gnp.tile([P, 1], F32, tag="vtmp")
                    nc.vector.tensor_mul(vtmp[:], gstat[:, 0:1], gstat[:, 0:1])
                    nc.vector.tensor_tensor(
                        out=vtmp[:], in0=gstat[:, 1:2], in1=vtmp[:],
                        op=mybir.AluOpType.subtract,
                    )
                    nc.scalar.activation(
                        out=vtmp[:], in_=vtmp[:],
                        func=mybir.ActivationFunctionType.Sqrt,
                        bias=eps_t[:], scale=1.0,
                    )
                    rstd = gnp.tile([P, 1], F32, tag="rstd")
                    nc.vector.reciprocal(out=rstd[:], in_=vtmp[:])
                    a_t = gnp.tile([P, 1], F32, tag="a_t")
                    nc.vector.tensor_mul(a_t[:], rstd[:], cvec_sb[:, 12 + ct:13 + ct])
                    d_t = gnp.tile([P, 1], F32, tag="d_t")
                    nc.vector.tensor_mul(d_t[:], gstat[:, 0:1], a_t[:])
                    nc.vector.tensor_tensor(
                        out=d_t[:], in0=cvec_sb[:, 16 + ct:17 + ct], in1=d_t[:],
                        op=mybir.AluOpType.subtract,
                    )
                    # apply split ACT/DVE so the last tile's apply is short
                    hdst = hn_f8[ct // 2]
                    nc.scalar.activation(
                        out=hdst[:, ct % 2, 0:2048],
                        in_=x_sb[ct][:, 0:2048],
                        func=IDENT, scale=a_t[:], bias=d_t[:],
                    )
                    nc.vector.tensor_scalar(
                        out=hdst[:, ct % 2, 2048:4096],
                        in0=x_sb[ct][:, 2048:4096],
                        scalar1=a_t[:], scalar2=d_t[:],
                        op0=MULT, op1=ADD,
                    )

            # deferred weight loads (after x so groupnorm owns DMA at t=0)
            wq_t = wq_h.rearrange("(t p) o -> t p o", p=P)
            wk_t = wk_h.rearrange("(t p) o -> t p o", p=P)
            wv_t = wv_h.rearrange("(t p) o -> t p o", p=P)
            wo_t = wo_h.rearrange("(t p) o -> t p o", p=P)
            for t in range(CP):
                for s in range(2):
                    nc.sync.dma_start(out=w_sb["wq"][t][:, s, :], in_=wq_t[2 * t + s])
                    nc.sync.dma_start(out=w_sb["wk"][t][:, s, :], in_=wk_t[2 * t + s])
                    nc.sync.dma_start(out=w_sb["wv"][t][:, s, :], in_=wv_t[2 * t + s])
                    nc.sync.dma_start(out=w_sb["wo"][t][:, s, :], in_=wo_t[2 * t + s])

            # =========== Phase B: k/q projections (fp8 DoubleRow) ===========
            # Per-engine psum pools (ACT and DVE drain their own rings in
            # parallel); 2-bank tiles pairing adjacent token slices of the
            # same out-channel block so the evac is one wide instruction.
            with tc.tile_pool(name="pjA", bufs=2, space="PSUM") as pjA, \
                 tc.tile_pool(name="pjD", bufs=2, space="PSUM") as pjD:

                def proj_pair(idx, wname, osl2, co, dst, bcol):
                    on_act = idx % 2 == 0
                    pool = pjA if on_act else pjD
                    ps = pool.tile([P, 1024], F32, tag="pj")
                    for h_ in range(2):
                        for t in range(CP):
                            nc.tensor.matmul(
                                ps[:, h_ * 512:(h_ + 1) * 512],
                                lhsT=w_sb[wname][t][:, :, co * P:(co + 1) * P],
                                rhs=hn_f8[t][:, :, (osl2 * 2 + h_) * 512:
                                             (osl2 * 2 + h_ + 1) * 512],
                                start=(t == 0), stop=(t == CP - 1),
                                perf_mode=DR,
                            )
                    if on_act:
                        nc.scalar.activation(out=dst, in_=ps[:], func=IDENT,
                                             bias=bcol, scale=1.0)
                    else:
                        nc.vector.tensor_scalar_add(out=dst, in0=ps[:],
                                                    scalar1=bcol)

                ei = 0
                # k for all 4096 keys, (jsl2, co) order so early score tiles
                # unblock first; then q
                for jsl2 in range(N // 1024):
                    for co in range(CT):
                        proj_pair(ei, "wk", jsl2, co,
                                  k_f8[co // 2][:, co % 2, jsl2 * 1024:(jsl2 + 1) * 1024],
                                  cvec_sb[:, 4 + co:5 + co])
                        ei += 1
                for isl2 in range(NQ // 1024):
                    for co in range(CT):
                        proj_pair(ei, "wq", isl2, co,
                                  q_f8[co // 2][:, co % 2, isl2 * 1024:(isl2 + 1) * 1024],
                                  cvec_sb[:, 0 + co:1 + co])
                        ei += 1

            # =========== Phase C: attention pipeline ===========
            # pT is double-buffered across strips so the ACT exp stream never
            # waits for consumers. Strip st's l and h matmuls run inside
            # strip st+1's score window; h accumulates CB-MAJOR (one output
            # channel block at a time over all 16 resident pT pairs), which
            # needs only a 2-bank ping-pong instead of 4 held banks. The v
            # projection hides inside strip 0's window; its psum pool closes
            # before the h pools open so everything fits in 8 banks.
            with tc.tile_pool(name="scA", bufs=2, space="PSUM") as scA, \
                 tc.tile_pool(name="attn", bufs=1) as ap_, \
                 tc.tile_pool(name="lsb", bufs=2) as lsp, \
                 tc.tile_pool(name="outt", bufs=3) as otp:

                # two pT sets (strip parity)
                pT = [[ap_.tile([P, 2, 512], F8, tag=f"pT{s}_{j}",
                                name=f"pT{s}_{j}") for j in range(JP)]
                      for s in range(2)]

                def sc_slot(st, jp):
                    """One 2-bank score pair tile + its exp."""
                    i0 = st * 512
                    sc = scA.tile([P, 1024], F32, tag="scA",
                                  name=f"s{st}_{jp}")
                    for h_ in range(2):
                        for t in range(CP):
                            nc.tensor.matmul(
                                sc[:, h_ * 512:(h_ + 1) * 512],
                                lhsT=k_f8[t][:, :, (2 * jp + h_) * P:(2 * jp + h_ + 1) * P],
                                rhs=q_f8[t][:, :, i0:i0 + 512],
                                start=(t == 0), stop=(t == CP - 1),
                                perf_mode=DR,
                            )
                    nc.scalar.activation(
                        out=pT[st % 2][jp][:], in_=sc[:],
                        func=EXP, scale=SCALE, bias=ebias_t[:],
                    )

                def aux_v(pjV):
                    """v projection: matmuls on PE, scaled-copy evac on DVE
                    (bv folded into bo on the host)."""
                    for jp in range(JP):
                        ps = pjV.tile([P, 1024], F32, tag="pv", name=f"v{jp}")
                        for m in range(2):
                            for t in range(CP):
                                yield nc.tensor.matmul(
                                    ps[:, m * 512:(m + 1) * 512],
                                    lhsT=hn_f8[t][:, :, (2 * jp + m) * P:(2 * jp + m + 1) * P],
                                    rhs=w_sb["wv"][t][:],
                                    start=(t == 0), stop=(t == CP - 1),
                                    perf_mode=DR,
                                )
                        nc.vector.tensor_scalar_mul(out=v_f8[jp][:], in0=ps[:],
                                                    scalar1=V_SCALE)

                def aux_lh(st, hp, lpool):
                    """Deferred work for strip st (runs in strip st+1's
                    window): l-run, rl, rlb, then cb-major h runs with
                    normalized fp8 evacs."""
                    i0 = st * 512
                    pts = pT[st % 2]
                    lt = lpool.tile([1, 512], F32, tag="l", name=f"l{st}")
                    for jp in range(JP):
                        yield nc.tensor.matmul(
                            lt[:], lhsT=ones_f8[:, :, 0:1], rhs=pts[jp][:],
                            start=(jp == 0), stop=(jp == JP - 1),
                            perf_mode=DR,
                        )
                    rl1 = lsp.tile([1, 512], F32, tag="rl1", name=f"rl1{st}")
                    nc.vector.reciprocal(out=rl1[:], in_=lt[:])
                    rlb = lsp.tile([P, 512], F32, tag="rlb", name=f"rlb{st}")
                    nc.gpsimd.partition_broadcast(rlb[:], rl1[:])
                    for cb in range(CT):
                        hps = hp.tile([P, 512], F32, tag="h",
                                      name=f"hps{st}_{cb}")
                        for jp in range(JP):
                            yield nc.tensor.matmul(
                                hps[:],
                                lhsT=v_f8[jp][:, :, cb * P:(cb + 1) * P],
                                rhs=pts[jp][:],
                                start=(jp == 0), stop=(jp == JP - 1),
                                perf_mode=DR,
                            )
                        nc.vector.tensor_mul(
                            hT_f8[cb // 2][:, cb % 2, i0:i0 + 512],
                            hps[:], rlb[:],
                        )

                def strip_out(st, hp):
                    """out-projection + bias + residual + store."""
                    i0 = st * 512
                    for co in range(CT):
                        ps = hp.tile([P, 512], F32, tag="h", name=f"op{st}_{co}")
                        for t in range(CP):
                            nc.tensor.matmul(
                                ps[:],
                                lhsT=w_sb["wo"][t][:, :, co * P:(co + 1) * P],
                                rhs=hT_f8[t][:, :, i0:i0 + 512],
                                start=(t == 0), stop=(t == CP - 1),
                                perf_mode=DR,
                            )
                        ot = otp.tile([P, 512], F32, tag="ot")
                        nc.vector.scalar_tensor_tensor(
                            out=ot[:], in0=ps[:], scalar=cvec_sb[:, 8 + co:9 + co],
                            in1=x_sb[co][:, i0:i0 + 512], op0=ADD, op1=ADD,
                        )
                        nc.sync.dma_start(
                            out=out_t[co][:, i0:i0 + 512], in_=ot[:]
                        )

                def weave(st, aux_gen):
                    """Emit strip st's 16 score slots with ~5 aux PE ops
                    between consecutive slots."""
                    for jp in range(JP):
                        sc_slot(st, jp)
                        if aux_gen is not None:
                            for _ in range(6 if st == 0 else 5):
                                if next(aux_gen, None) is None:
                                    aux_gen = None
                                    break
                    while aux_gen is not None and next(aux_gen, None) is not None:
                        pass

                # strip 0 (v hides in its window; pjV closes right after)
                pjV_cm = tc.tile_pool(name="pjV", bufs=2, space="PSUM")
                pjV = pjV_cm.__enter__()
                weave(0, aux_v(pjV))
                pjV_cm.__exit__(None, None, None)

                hp_cm = tc.tile_pool(name="hacc", bufs=2, space="PSUM")
                hp = hp_cm.__enter__()
                lp_cm = tc.tile_pool(name="lps", bufs=1, space="PSUM")
                lpool = lp_cm.__enter__()

                for st in range(1, NSTRIP):
                    weave(st, aux_lh(st - 1, hp, lpool))
                    strip_out(st - 1, hp)
                # drain: last strip's deferred work
                for _ in aux_lh(NSTRIP - 1, hp, lpool):
                    pass
                strip_out(NSTRIP - 1, hp)

                lp_cm.__exit__(None, None, None)
                hp_cm.__exit__(None, None, None)

            if debug:
                for t in range(CP):
                    nc.sync.dma_start(out=dbg["hn"][t], in_=hn_f8[t][:])
                    nc.sync.dma_start(out=dbg["q"][t], in_=q_f8[t][:])
                    nc.sync.dma_start(out=dbg["k"][t], in_=k_f8[t][:])
                    nc.sync.dma_start(out=dbg["hT"][t], in_=hT_f8[t][:])
                for jp in range(JP):
                    nc.sync.dma_start(out=dbg["v"][jp], in_=v_f8[jp][:])

    nc.finalize()
    return nc


def kernel(**inputs):
    if "nc" not in _CACHE:
        _CACHE["nc"] = build_bass()
    nc = _CACHE["nc"]

    x = np.ascontiguousarray(np.asarray(inputs["x"], dtype=np.float32))
    B = x.shape[0]
    xf = x.reshape(B, C, N)

    def f8T(w, scale=1.0):
        return np.ascontiguousarray(
            (np.asarray(w, dtype=np.float32).T * scale).astype(
                ml_dtypes.float8_e4m3)
        )

    # softmax weights sum to 1, so the v bias rides through attention:
    # h = p@(v0+bv)/l = p@v0/l + bv  =>  fold wo@bv into bo (exact, fp32)
    wo32 = np.asarray(inputs["wo"], np.float32)
    bo_eff = (np.asarray(inputs["bo"], np.float32)
              + wo32 @ np.asarray(inputs["bv"], np.float32))

    def colsT(v):
        return np.asarray(v, np.float32).reshape(CT, P).T

    g8_np = np.zeros((P, 8), np.float32)
    for c in range(P):
        g8_np[c, c // 16] = 1.0 / 16
    cvec = np.concatenate([
        colsT(inputs["bq"]), colsT(inputs["bk"]), colsT(bo_eff),
        colsT(inputs["norm_g"]), colsT(inputs["norm_b"]), g8_np,
    ], axis=1)

    shared = {
        "wqT": f8T(inputs["wq"]), "wkT": f8T(inputs["wk"]),
        "wvT": f8T(inputs["wv"]), "woT": f8T(inputs["wo"], 1.0 / V_SCALE),
        "cvec": np.ascontiguousarray(cvec, dtype=np.float32),
    }

    in_maps = []
    for core in range(2 * B):
        b, half = core // 2, core % 2
        xb = xf[b]
        if half:
            xb = np.concatenate([xb[:, NQ:], xb[:, :NQ]], axis=1)
        in_maps.append({"x": np.ascontiguousarray(xb), **shared})

    import os
    trace = bool(os.environ.get("BASS_KERNEL_TRACE"))
    res = run_bass_kernel_spmd(
        nc, in_maps, core_ids=list(range(2 * B)), trace=trace,
        trace_cores=list(range(2 * B)) if trace else None,
    )
    _CACHE["last_results"] = res

    out = np.empty((B, C, N), np.float32)
    for core in range(2 * B):
        b, half = core // 2, core % 2
        out[b][:, half * NQ:(half + 1) * NQ] = res.results[core]["out"]
    return out.reshape(B, C, 64, 64)
